# revision 10
# baseline (speedup 1.0000x reference)
"""BFP (block floating point) fake-quant kernel for Trainium2.

Reference op (DMXQuantizer): per 64-element block along the last dim,
  maxabs = max(|x_block|)
  e      = floor(log2(maxabs))
  delta  = 2^(e - (bits-2))          # bits = 8 -> delta = 2^(e-6)
  q      = clip(round(x/delta), -128, 127) * delta     (round = RNE)
  q      = 0 where maxabs == 0

Implementation notes:
- delta is a power of two, derived bit-exactly from maxabs's exponent field:
    delta_bits = (maxabs_bits & 0x7F800000) - (6 << 23)
    inv_bits   = 0x7F000000 - delta_bits          # 1/delta, exact
- round-to-nearest-even via the (y + 1.5*2^23) - 1.5*2^23 trick (|y| < 128).
- y = x * (1/delta) is exact (power-of-2 scale), so RNE on y matches
  round(x/delta) exactly, including ties-to-even.
- y in (-128, 128) strictly, so the lower clip never binds; only min(r, 127)
  is needed, fused with the delta scale-back via scalar_tensor_tensor.
- every output value is an integer in [-128, 127] times a power of two ->
  exactly representable in bfloat16. Output is written as bf16 (halves the
  store traffic); the host upcast to fp32 is lossless.
- sharding: x[4096, 8192] row-sharded over 8 cores, 512 rows each; 64-elem
  blocks live on the last dim so every block is core-local.
"""

import sys

sys.path.insert(0, "/opt/trn_rl_repo")

import numpy as np

import concourse.bacc as bacc
import concourse.bass as bass  # noqa: F401  (AP types)
import concourse.tile as tile
from concourse import mybir
from concourse import bass_utils

N_CORES = 8
ROWS, COLS = 4096, 8192
SHARD_ROWS = ROWS // N_CORES  # 512
BLOCK = 64
P = 128  # SBUF partitions

_RNE_C = 12582912.0  # 1.5 * 2^23: add/sub forces round-to-nearest-even
_EXP_MASK = 0x7F800000
_DELTA_BIAS = 0x03000000  # 6 << 23: delta = 2^(e-6)
_INV_CONST = 0x7F000000  # inv_bits = 0x7F000000 - delta_bits


def build_quant_kernel(
    rows: int = SHARD_ROWS,
    cols: int = COLS,
    out_dtype=mybir.dt.bfloat16,
    n_tiles: int | None = None,
    sub_splits: int | None = None,
    reps: int = 1,
):
    """One-core BFP quant program: x[rows, cols] f32 -> out[rows, cols] bf16.

    The shard is processed as `n_tiles` SBUF-resident tiles of shape
    [128, (rows/n_tiles/128)*cols]; partition p of tile t holds DRAM rows
    {t*rows/n_tiles + j*128 + p}. Each tile's load is split into `sub_splits`
    DMAs / compute chunks for pipelining. Total DMA instruction count is kept
    low on purpose: each HWDGE dma_start takes one of 8 round-robin semaphore
    lanes, and the kernel-tail drain can only encode ~8 sync waits.
    """
    if n_tiles is None:
        n_tiles = max(1, rows // (2 * P))  # default: half-shard tiles
    jt = rows // (P * n_tiles)  # DRAM row groups per tile
    assert rows == P * n_tiles * jt
    if sub_splits is None:
        sub_splits = jt
    assert jt % sub_splits == 0 or sub_splits % jt == 0
    ft = jt * cols  # free elems per tile

    nc = bacc.Bacc("TRN2")
    x = nc.dram_tensor("x", [rows, cols], mybir.dt.float32, kind="ExternalInput")
    out = nc.dram_tensor("out", [rows, cols], out_dtype, kind="ExternalOutput")

    with tile.TileContext(nc) as tc:
        with (
            tc.tile_pool(name="xp", bufs=min(n_tiles, 2)) as xp,
            tc.tile_pool(name="qp", bufs=min(n_tiles, 2)) as qp,
            tc.tile_pool(name="sp", bufs=min(n_tiles, 2)) as sp,
        ):
            rep_ctx = tc.For_i(0, reps, 1) if reps > 1 else None
            if rep_ctx is not None:
                rep_ctx.__enter__()
            for t in range(n_tiles):
                r0 = t * P * jt
                x_t = xp.tile([P, ft], mybir.dt.float32)
                q = qp.tile([P, ft], out_dtype)
                # split the tile into sub-chunks for load/compute pipelining
                sub = ft // sub_splits
                x_dram = x[r0 : r0 + P * jt, :].rearrange("(j p) c -> p j c", p=P)
                x_t3 = x_t.rearrange("p (j c) -> p j c", j=jt)
                assert sub % cols == 0 or cols % sub == 0
                for s in range(sub_splits):
                    if sub >= cols:
                        js = sub // cols
                        nc.sync.dma_start(
                            out=x_t3[:, s * js : (s + 1) * js, :],
                            in_=x_dram[:, s * js : (s + 1) * js, :],
                        )
                    else:
                        j0, c0 = (s * sub) // cols, (s * sub) % cols
                        nc.sync.dma_start(
                            out=x_t3[:, j0, c0 : c0 + sub],
                            in_=x_dram[:, j0, c0 : c0 + sub],
                        )
                for s in range(sub_splits):
                    xs = x_t[:, s * sub : (s + 1) * sub]
                    xb = xs.rearrange("p (b i) -> p b i", i=BLOCK)
                    nblk = sub // BLOCK
                    # maxabs per 64-block: one fused |.|+max reduce
                    m = sp.tile([P, nblk], mybir.dt.float32, name="m", tag="m", bufs=4)
                    nc.vector.tensor_reduce(
                        out=m,
                        in_=xb,
                        axis=mybir.AxisListType.X,
                        op=mybir.AluOpType.max,
                        apply_absolute_value=True,
                    )
                    # delta_bits = (bits(maxabs) & EXP_MASK) - (6 << 23)
                    # (walrus rejects bitwise+arith in one tensor_scalar)
                    db = sp.tile([P, nblk], mybir.dt.int32, name="db", tag="db", bufs=4)
                    nc.vector.tensor_scalar(
                        out=db,
                        in0=m.bitcast(mybir.dt.int32),
                        scalar1=_EXP_MASK,
                        scalar2=None,
                        op0=mybir.AluOpType.bitwise_and,
                    )
                    nc.vector.tensor_scalar(
                        out=db,
                        in0=db,
                        scalar1=_DELTA_BIAS,
                        scalar2=None,
                        op0=mybir.AluOpType.subtract,
                    )
                    # inv_bits = 0x7F000000 - delta_bits  (== bits of 1/delta)
                    ib = sp.tile([P, nblk], mybir.dt.int32, name="ib", tag="ib", bufs=4)
                    nc.vector.tensor_scalar(
                        out=ib,
                        in0=db,
                        scalar1=-1,
                        scalar2=_INV_CONST,
                        op0=mybir.AluOpType.mult,
                        op1=mybir.AluOpType.add,
                    )
                    inv_b = (
                        ib.bitcast(mybir.dt.float32)
                        .unsqueeze(2)
                        .broadcast_to((P, nblk, BLOCK))
                    )
                    delta_b = (
                        db.bitcast(mybir.dt.float32)
                        .unsqueeze(2)
                        .broadcast_to((P, nblk, BLOCK))
                    )
                    # y = x / delta (exact)
                    nc.vector.tensor_tensor(
                        out=xb, in0=xb, in1=inv_b, op=mybir.AluOpType.mult
                    )
                    # r = RNE(y)
                    nc.vector.tensor_scalar(
                        out=xs,
                        in0=xs,
                        scalar1=_RNE_C,
                        scalar2=_RNE_C,
                        op0=mybir.AluOpType.add,
                        op1=mybir.AluOpType.subtract,
                    )
                    # q = min(r, 127) * delta
                    qs = q[:, s * sub : (s + 1) * sub]
                    qb = qs.rearrange("p (b i) -> p b i", i=BLOCK)
                    nc.vector.scalar_tensor_tensor(
                        out=qb,
                        in0=xb,
                        scalar=127.0,
                        in1=delta_b,
                        op0=mybir.AluOpType.min,
                        op1=mybir.AluOpType.mult,
                    )
                out_dram = out[r0 : r0 + P * jt, :].rearrange("(j p) c -> p j c", p=P)
                nc.sync.dma_start(
                    out=out_dram, in_=q.rearrange("p (j c) -> p j c", j=jt)
                )
            if rep_ctx is not None:
                rep_ctx.__exit__(None, None, None)
    nc.compile()
    return nc


_nc_cache = {}


def _get_nc():
    if "nc" not in _nc_cache:
        _nc_cache["nc"] = build_quant_kernel()
    return _nc_cache["nc"]


def _run(x_np: np.ndarray, trace: bool = False):
    nc = _get_nc()
    shards = [
        np.ascontiguousarray(x_np[i * SHARD_ROWS : (i + 1) * SHARD_ROWS])
        for i in range(N_CORES)
    ]
    res = bass_utils.run_bass_kernel_spmd(
        nc,
        [{"x": s} for s in shards],
        core_ids=list(range(N_CORES)),
        trace=trace,
    )
    out = np.concatenate(
        [np.asarray(r["out"]).astype(np.float32) for r in res.results], axis=0
    )
    return out, res


def kernel(x, bits):
    assert int(np.asarray(bits)) == 8
    x_np = np.asarray(x, dtype=np.float32)
    assert x_np.shape == (ROWS, COLS)
    out, _ = _run(x_np, trace=False)
    return out


def bench_hw_ns(x_np, r_lo=1, r_hi=101, n_times=5):
    """Estimate HW exec time of one kernel invocation by on-device repeat
    loops: wall(r_hi reps) - wall(r_lo reps) removes the constant axon RPC +
    host<->device transfer overhead. Returns (ns_per_iter, details)."""
    import time

    shards = [
        np.ascontiguousarray(x_np[i * SHARD_ROWS : (i + 1) * SHARD_ROWS])
        for i in range(N_CORES)
    ]
    in_maps = [{"x": s} for s in shards]
    walls = {}
    for reps in (r_lo, r_hi):
        nc = build_quant_kernel(reps=reps)
        ts = []
        for it in range(n_times):
            t0 = time.monotonic()
            bass_utils.run_bass_kernel_spmd(
                nc, in_maps, core_ids=list(range(N_CORES))
            )
            ts.append(time.monotonic() - t0)
        walls[reps] = sorted(ts)
    # first run of each includes compile; use median of the rest
    lo = np.median(walls[r_lo][: max(1, n_times - 1)])
    hi = np.median(walls[r_hi][: max(1, n_times - 1)])
    ns = (hi - lo) / (r_hi - r_lo) * 1e9
    return ns, walls


# revision 14
# speedup vs baseline: 4.0814x; 4.0814x over previous
"""BFP (block floating point) fake-quant kernel for Trainium2.

Reference op (DMXQuantizer): per 64-element block along the last dim,
  maxabs = max(|x_block|)
  e      = floor(log2(maxabs))
  delta  = 2^(e - (bits-2))          # bits = 8 -> delta = 2^(e-6)
  q      = clip(round(x/delta), -128, 127) * delta     (round = RNE)
  q      = 0 where maxabs == 0

Implementation notes (v3 pipeline, bit-exact vs the fp32 jax reference):
- maxabs only matters through its exponent. ACT extracts |trunc_bf16(x)| from
  the fp32 high halves (strided bf16 view + Abs); truncation is monotone and
  exponent-preserving, so a bf16 max tree on DVE (2x mode) yields per-block
  maxima with the exact exponent.
- delta is a power of two, derived bit-exactly from the exponent field:
    delta_bits = (maxabs_bits & 0x7F800000) - (6 << 23)
    inv_bits   = 0x7F000000 - delta_bits          # 1/delta, exact
- y = x * (1/delta) is exact (power-of-2 scale). The fp32->int8 output cast
  on the DVE is HW-verified round-to-nearest-even + saturation, which IS
  clip(round(y), -128, 127) in a single op (y in (-128,128) strictly, and
  RNE(127.5..128) saturates to 127 exactly like the reference's clip).
- q = y8 * delta is exact; every output is an integer in [-128, 127] times a
  power of two -> exactly representable in bfloat16. Output is stored as bf16
  (halves store traffic); the host upcast to fp32 is lossless.
- sharding: x[4096, 8192] row-sharded over 8 cores, 512 rows each; 64-elem
  blocks live on the last dim so every block is core-local.
- engines: ACT ~28us, DVE ~85us (tree + 2 full TT passes + small bit ops),
  DMA 24 MiB/core; modeled single-shot ~120us/core, loop-measured ~116us.
"""

import sys

sys.path.insert(0, "/opt/trn_rl_repo")

import numpy as np

import concourse.bacc as bacc
import concourse.bass as bass  # noqa: F401  (AP types)
import concourse.tile as tile
from concourse import mybir
from concourse import bass_utils

N_CORES = 8
ROWS, COLS = 4096, 8192
SHARD_ROWS = ROWS // N_CORES  # 512
BLOCK = 64
P = 128  # SBUF partitions

_RNE_C = 12582912.0  # 1.5 * 2^23: add/sub forces round-to-nearest-even
_EXP_MASK = 0x7F800000
_DELTA_BIAS = 0x03000000  # 6 << 23: delta = 2^(e-6)
_INV_CONST = 0x7F000000  # inv_bits = 0x7F000000 - delta_bits


def build_quant_kernel(
    rows: int = SHARD_ROWS,
    cols: int = COLS,
    out_dtype=mybir.dt.bfloat16,
    n_tiles: int | None = None,
    sub_splits: int | None = None,
    reps: int = 1,
):
    """One-core BFP quant program: x[rows, cols] f32 -> out[rows, cols] bf16.

    The shard is processed as `n_tiles` SBUF-resident tiles of shape
    [128, (rows/n_tiles/128)*cols]; partition p of tile t holds DRAM rows
    {t*rows/n_tiles + j*128 + p}. Each tile's load is split into `sub_splits`
    DMAs / compute chunks for pipelining. Total DMA instruction count is kept
    low on purpose: each HWDGE dma_start takes one of 8 round-robin semaphore
    lanes, and the kernel-tail drain can only encode ~8 sync waits.
    """
    if n_tiles is None:
        n_tiles = max(1, rows // (2 * P))  # default: half-shard tiles
    jt = rows // (P * n_tiles)  # DRAM row groups per tile
    assert rows == P * n_tiles * jt
    if sub_splits is None:
        sub_splits = jt
    assert jt % sub_splits == 0 or sub_splits % jt == 0
    ft = jt * cols  # free elems per tile

    nc = bacc.Bacc("TRN2")
    x = nc.dram_tensor("x", [rows, cols], mybir.dt.float32, kind="ExternalInput")
    out = nc.dram_tensor("out", [rows, cols], out_dtype, kind="ExternalOutput")

    with tile.TileContext(nc) as tc:
        with (
            tc.tile_pool(name="xp", bufs=min(n_tiles, 2)) as xp,
            tc.tile_pool(name="qp", bufs=min(n_tiles, 2)) as qp,
            tc.tile_pool(name="sp", bufs=min(n_tiles, 2)) as sp,
        ):
            rep_ctx = tc.For_i(0, reps, 1) if reps > 1 else None
            if rep_ctx is not None:
                rep_ctx.__enter__()
            for t in range(n_tiles):
                r0 = t * P * jt
                x_t = xp.tile([P, ft], mybir.dt.float32)
                q = qp.tile([P, ft], out_dtype)
                # split the tile into sub-chunks for load/compute pipelining
                sub = ft // sub_splits
                x_dram = x[r0 : r0 + P * jt, :].rearrange("(j p) c -> p j c", p=P)
                x_t3 = x_t.rearrange("p (j c) -> p j c", j=jt)
                assert sub % cols == 0 or cols % sub == 0
                for s in range(sub_splits):
                    if sub >= cols:
                        js = sub // cols
                        nc.sync.dma_start(
                            out=x_t3[:, s * js : (s + 1) * js, :],
                            in_=x_dram[:, s * js : (s + 1) * js, :],
                        )
                    else:
                        j0, c0 = (s * sub) // cols, (s * sub) % cols
                        nc.sync.dma_start(
                            out=x_t3[:, j0, c0 : c0 + sub],
                            in_=x_dram[:, j0, c0 : c0 + sub],
                        )
                for s in range(sub_splits):
                    xs = x_t[:, s * sub : (s + 1) * sub]
                    xb = xs.rearrange("p (b i) -> p b i", i=BLOCK)
                    nblk = sub // BLOCK
                    # maxabs per 64-block: one fused |.|+max reduce
                    m = sp.tile([P, nblk], mybir.dt.float32, name="m", tag="m", bufs=4)
                    nc.vector.tensor_reduce(
                        out=m,
                        in_=xb,
                        axis=mybir.AxisListType.X,
                        op=mybir.AluOpType.max,
                        apply_absolute_value=True,
                    )
                    # delta_bits = (bits(maxabs) & EXP_MASK) - (6 << 23)
                    # (walrus rejects bitwise+arith in one tensor_scalar)
                    db = sp.tile([P, nblk], mybir.dt.int32, name="db", tag="db", bufs=4)
                    nc.vector.tensor_scalar(
                        out=db,
                        in0=m.bitcast(mybir.dt.int32),
                        scalar1=_EXP_MASK,
                        scalar2=None,
                        op0=mybir.AluOpType.bitwise_and,
                    )
                    nc.vector.tensor_scalar(
                        out=db,
                        in0=db,
                        scalar1=_DELTA_BIAS,
                        scalar2=None,
                        op0=mybir.AluOpType.subtract,
                    )
                    # inv_bits = 0x7F000000 - delta_bits  (== bits of 1/delta)
                    ib = sp.tile([P, nblk], mybir.dt.int32, name="ib", tag="ib", bufs=4)
                    nc.vector.tensor_scalar(
                        out=ib,
                        in0=db,
                        scalar1=-1,
                        scalar2=_INV_CONST,
                        op0=mybir.AluOpType.mult,
                        op1=mybir.AluOpType.add,
                    )
                    inv_b = (
                        ib.bitcast(mybir.dt.float32)
                        .unsqueeze(2)
                        .broadcast_to((P, nblk, BLOCK))
                    )
                    delta_b = (
                        db.bitcast(mybir.dt.float32)
                        .unsqueeze(2)
                        .broadcast_to((P, nblk, BLOCK))
                    )
                    # y = x / delta (exact)
                    nc.vector.tensor_tensor(
                        out=xb, in0=xb, in1=inv_b, op=mybir.AluOpType.mult
                    )
                    # r = RNE(y)
                    nc.vector.tensor_scalar(
                        out=xs,
                        in0=xs,
                        scalar1=_RNE_C,
                        scalar2=_RNE_C,
                        op0=mybir.AluOpType.add,
                        op1=mybir.AluOpType.subtract,
                    )
                    # q = min(r, 127) * delta
                    qs = q[:, s * sub : (s + 1) * sub]
                    qb = qs.rearrange("p (b i) -> p b i", i=BLOCK)
                    nc.vector.scalar_tensor_tensor(
                        out=qb,
                        in0=xb,
                        scalar=127.0,
                        in1=delta_b,
                        op0=mybir.AluOpType.min,
                        op1=mybir.AluOpType.mult,
                    )
                out_dram = out[r0 : r0 + P * jt, :].rearrange("(j p) c -> p j c", p=P)
                nc.sync.dma_start(
                    out=out_dram, in_=q.rearrange("p (j c) -> p j c", j=jt)
                )
            if rep_ctx is not None:
                rep_ctx.__exit__(None, None, None)
    nc.compile()
    return nc




def build_quant_v3(
    rows: int = SHARD_ROWS,
    cols: int = COLS,
    reps: int = 1,
    xp_bufs: int = 2,
    gpsimd_tt2: bool = False,
):
    """v3: per-tile [128, 8192] pipeline.
    - ACT extracts |trunc_bf16(x)| from the fp32 high halves (exponent-exact,
      truncation is monotone -> exponent(max) is preserved).
    - DVE folds a bf16 max tree (2x mode) to per-block maxima.
    - delta/inv derived bit-exactly from the exponent field (int32 ops).
    - TT1: y8 = x * inv -> int8 output (HW-verified RNE + saturation does the
      round AND the clip to [-128, 127] in the cast).
    - TT2: q = y8 * delta -> bf16 (exact).
    """
    nc = bacc.Bacc("TRN2")
    x = nc.dram_tensor("x", [rows, cols], mybir.dt.float32, kind="ExternalInput")
    out = nc.dram_tensor("out", [rows, cols], mybir.dt.bfloat16, kind="ExternalOutput")
    n_tiles = rows // P
    nblk = cols // BLOCK

    with tile.TileContext(nc) as tc:
        with (
            tc.tile_pool(name="xp", bufs=xp_bufs) as xp,
            tc.tile_pool(name="qp", bufs=2) as qp,
            tc.tile_pool(name="hp", bufs=2) as hp,
            tc.tile_pool(name="tp", bufs=2) as tp,
            tc.tile_pool(name="sp", bufs=4) as sp,
        ):
            rep_ctx = tc.For_i(0, reps, 1) if reps > 1 else None
            if rep_ctx is not None:
                rep_ctx.__enter__()
            for t in range(n_tiles):
                x_t = xp.tile([P, cols], mybir.dt.float32, name="x_t", tag="x_t")
                nc.sync.dma_start(out=x_t, in_=x[t * P : (t + 1) * P, :])
                xb = x_t.rearrange("p (b i) -> p b i", i=BLOCK)

                # |trunc_bf16(x)|: strided high halves, Abs on ACT
                habs = hp.tile([P, cols], mybir.dt.bfloat16, name="habs", tag="habs")
                nc.scalar.activation(
                    out=habs,
                    in_=x_t.bitcast(mybir.dt.bfloat16)[:, 1::2],
                    func=mybir.ActivationFunctionType.Abs,
                )
                # bf16 max tree: 64 -> 32 -> 16 -> 8 -> 4 -> 2 -> 1 per block
                h3 = habs.rearrange("p (b i) -> p b i", i=BLOCK)
                t1 = tp.tile([P, nblk, 32], mybir.dt.bfloat16, name="t1", tag="t1")
                t2 = tp.tile([P, nblk, 16], mybir.dt.bfloat16, name="t2", tag="t2")
                m = sp.tile([P, nblk], mybir.dt.bfloat16, name="m", tag="m")
                mx = mybir.AluOpType.max
                nc.vector.tensor_tensor(out=t1, in0=h3[:, :, 0:32], in1=h3[:, :, 32:64], op=mx)
                nc.vector.tensor_tensor(out=t2, in0=t1[:, :, 0:16], in1=t1[:, :, 16:32], op=mx)
                nc.vector.tensor_tensor(out=t1[:, :, 0:8], in0=t2[:, :, 0:8], in1=t2[:, :, 8:16], op=mx)
                nc.vector.tensor_tensor(out=t2[:, :, 0:4], in0=t1[:, :, 0:4], in1=t1[:, :, 4:8], op=mx)
                nc.vector.tensor_tensor(out=t1[:, :, 0:2], in0=t2[:, :, 0:2], in1=t2[:, :, 2:4], op=mx)
                nc.vector.tensor_tensor(
                    out=m.rearrange("p (b i) -> p b i", i=1),
                    in0=t1[:, :, 0:1], in1=t1[:, :, 1:2], op=mx,
                )

                # delta/inv from the exponent field (proven int32 path)
                mf = sp.tile([P, nblk], mybir.dt.float32, name="mf", tag="mf")
                nc.vector.tensor_copy(out=mf, in_=m)
                db = sp.tile([P, nblk], mybir.dt.int32, name="db", tag="db")
                nc.vector.tensor_scalar(
                    out=db, in0=mf.bitcast(mybir.dt.int32),
                    scalar1=_EXP_MASK, scalar2=None, op0=mybir.AluOpType.bitwise_and,
                )
                nc.vector.tensor_scalar(
                    out=db, in0=db,
                    scalar1=_DELTA_BIAS, scalar2=None, op0=mybir.AluOpType.subtract,
                )
                ib = sp.tile([P, nblk], mybir.dt.int32, name="ib", tag="ib")
                nc.vector.tensor_scalar(
                    out=ib, in0=db, scalar1=-1, scalar2=_INV_CONST,
                    op0=mybir.AluOpType.mult, op1=mybir.AluOpType.add,
                )
                inv_b = (
                    ib.bitcast(mybir.dt.float32).unsqueeze(2)
                    .broadcast_to((P, nblk, BLOCK))
                )
                delta_b = (
                    db.bitcast(mybir.dt.float32).unsqueeze(2)
                    .broadcast_to((P, nblk, BLOCK))
                )
                # y8 = round/clip(x / delta) via saturating int8 cast;
                # habs is dead past the tree -> reuse its bytes for y8
                y8 = habs.bitcast(mybir.dt.int8)[:, 0:cols]
                y8b = y8.rearrange("p (b i) -> p b i", i=BLOCK)
                nc.vector.tensor_tensor(out=y8b, in0=xb, in1=inv_b, op=mybir.AluOpType.mult)
                # q = y8 * delta, exact in bf16
                q = qp.tile([P, cols], mybir.dt.bfloat16, name="q", tag="q")
                qb = q.rearrange("p (b i) -> p b i", i=BLOCK)
                eng2 = nc.gpsimd if gpsimd_tt2 else nc.vector
                eng2.tensor_tensor(out=qb, in0=y8b, in1=delta_b, op=mybir.AluOpType.mult)
                nc.sync.dma_start(out=out[t * P : (t + 1) * P, :], in_=q)
            if rep_ctx is not None:
                rep_ctx.__exit__(None, None, None)
    nc.compile()
    return nc


_nc_cache = {}


def _get_nc():
    if "nc" not in _nc_cache:
        _nc_cache["nc"] = build_quant_v3(xp_bufs=3)
    return _nc_cache["nc"]


def _run(x_np: np.ndarray, trace: bool = False):
    nc = _get_nc()
    shards = [
        np.ascontiguousarray(x_np[i * SHARD_ROWS : (i + 1) * SHARD_ROWS])
        for i in range(N_CORES)
    ]
    res = bass_utils.run_bass_kernel_spmd(
        nc,
        [{"x": s} for s in shards],
        core_ids=list(range(N_CORES)),
        trace=trace,
    )
    out = np.concatenate(
        [np.asarray(r["out"]).astype(np.float32) for r in res.results], axis=0
    )
    return out, res


def kernel(x, bits):
    assert int(np.asarray(bits)) == 8
    x_np = np.asarray(x, dtype=np.float32)
    assert x_np.shape == (ROWS, COLS)
    out, _ = _run(x_np, trace=False)
    return out


def bench_hw_ns(x_np, r_lo=1, r_hi=2001, n_times=5):
    """Estimate HW exec time of one kernel invocation by on-device repeat
    loops: wall(r_hi reps) - wall(r_lo reps) removes the constant axon RPC +
    host<->device transfer overhead. Returns (ns_per_iter, details)."""
    import time

    shards = [
        np.ascontiguousarray(x_np[i * SHARD_ROWS : (i + 1) * SHARD_ROWS])
        for i in range(N_CORES)
    ]
    in_maps = [{"x": s} for s in shards]
    walls = {}
    for reps in (r_lo, r_hi):
        nc = build_quant_v3(xp_bufs=3, reps=reps) if reps > 1 else _get_nc()
        ts = []
        for it in range(n_times):
            t0 = time.monotonic()
            bass_utils.run_bass_kernel_spmd(
                nc, in_maps, core_ids=list(range(N_CORES))
            )
            ts.append(time.monotonic() - t0)
        walls[reps] = sorted(ts)
    # first run of each includes compile; use median of the rest
    lo = np.median(walls[r_lo][: max(1, n_times - 1)])
    hi = np.median(walls[r_hi][: max(1, n_times - 1)])
    ns = (hi - lo) / (r_hi - r_lo) * 1e9
    return ns, walls


# revision 15
# speedup vs baseline: 17.0867x; 4.1865x over previous
"""BFP (block floating point) fake-quant kernel for Trainium2.

Reference op (DMXQuantizer): per 64-element block along the last dim,
  maxabs = max(|x_block|)
  e      = floor(log2(maxabs))
  delta  = 2^(e - (bits-2))          # bits = 8 -> delta = 2^(e-6)
  q      = clip(round(x/delta), -128, 127) * delta     (round = RNE)
  q      = 0 where maxabs == 0

Implementation notes (v3 pipeline, bit-exact vs the fp32 jax reference):
- maxabs only matters through its exponent. ACT extracts |trunc_bf16(x)| from
  the fp32 high halves (strided bf16 view + Abs); truncation is monotone and
  exponent-preserving, so a bf16 max tree on DVE (2x mode) yields per-block
  maxima with the exact exponent.
- delta is a power of two, derived bit-exactly from the exponent field:
    delta_bits = (maxabs_bits & 0x7F800000) - (6 << 23)
    inv_bits   = 0x7F000000 - delta_bits          # 1/delta, exact
- y = x * (1/delta) is exact (power-of-2 scale). The fp32->int8 output cast
  on the DVE is HW-verified round-to-nearest-even + saturation, which IS
  clip(round(y), -128, 127) in a single op (y in (-128,128) strictly, and
  RNE(127.5..128) saturates to 127 exactly like the reference's clip).
- q = y8 * delta is exact; every output is an integer in [-128, 127] times a
  power of two -> exactly representable in bfloat16. Output is stored as bf16
  (halves store traffic); the host upcast to fp32 is lossless.
- sharding: x[4096, 8192] row-sharded over 8 cores, 512 rows each; 64-elem
  blocks live on the last dim so every block is core-local.
- engines: ACT ~28us, DVE ~85us (tree + 2 full TT passes + small bit ops),
  DMA 24 MiB/core; modeled single-shot ~120us/core, loop-measured ~116us.
"""

import sys

sys.path.insert(0, "/opt/trn_rl_repo")

import numpy as np

import concourse.bacc as bacc
import concourse.bass as bass  # noqa: F401  (AP types)
import concourse.tile as tile
from concourse import mybir
from concourse import bass_utils

N_CORES = 8
ROWS, COLS = 4096, 8192
SHARD_ROWS = ROWS // N_CORES  # 512
BLOCK = 64
P = 128  # SBUF partitions

_RNE_C = 12582912.0  # 1.5 * 2^23: add/sub forces round-to-nearest-even
_EXP_MASK = 0x7F800000
_DELTA_BIAS = 0x03000000  # 6 << 23: delta = 2^(e-6)
_INV_CONST = 0x7F000000  # inv_bits = 0x7F000000 - delta_bits


def build_quant_kernel(
    rows: int = SHARD_ROWS,
    cols: int = COLS,
    out_dtype=mybir.dt.bfloat16,
    n_tiles: int | None = None,
    sub_splits: int | None = None,
    reps: int = 1,
):
    """One-core BFP quant program: x[rows, cols] f32 -> out[rows, cols] bf16.

    The shard is processed as `n_tiles` SBUF-resident tiles of shape
    [128, (rows/n_tiles/128)*cols]; partition p of tile t holds DRAM rows
    {t*rows/n_tiles + j*128 + p}. Each tile's load is split into `sub_splits`
    DMAs / compute chunks for pipelining. Total DMA instruction count is kept
    low on purpose: each HWDGE dma_start takes one of 8 round-robin semaphore
    lanes, and the kernel-tail drain can only encode ~8 sync waits.
    """
    if n_tiles is None:
        n_tiles = max(1, rows // (2 * P))  # default: half-shard tiles
    jt = rows // (P * n_tiles)  # DRAM row groups per tile
    assert rows == P * n_tiles * jt
    if sub_splits is None:
        sub_splits = jt
    assert jt % sub_splits == 0 or sub_splits % jt == 0
    ft = jt * cols  # free elems per tile

    nc = bacc.Bacc("TRN2")
    x = nc.dram_tensor("x", [rows, cols], mybir.dt.float32, kind="ExternalInput")
    out = nc.dram_tensor("out", [rows, cols], out_dtype, kind="ExternalOutput")

    with tile.TileContext(nc) as tc:
        with (
            tc.tile_pool(name="xp", bufs=min(n_tiles, 2)) as xp,
            tc.tile_pool(name="qp", bufs=min(n_tiles, 2)) as qp,
            tc.tile_pool(name="sp", bufs=min(n_tiles, 2)) as sp,
        ):
            rep_ctx = tc.For_i(0, reps, 1) if reps > 1 else None
            if rep_ctx is not None:
                rep_ctx.__enter__()
            for t in range(n_tiles):
                r0 = t * P * jt
                x_t = xp.tile([P, ft], mybir.dt.float32)
                q = qp.tile([P, ft], out_dtype)
                # split the tile into sub-chunks for load/compute pipelining
                sub = ft // sub_splits
                x_dram = x[r0 : r0 + P * jt, :].rearrange("(j p) c -> p j c", p=P)
                x_t3 = x_t.rearrange("p (j c) -> p j c", j=jt)
                assert sub % cols == 0 or cols % sub == 0
                for s in range(sub_splits):
                    if sub >= cols:
                        js = sub // cols
                        nc.sync.dma_start(
                            out=x_t3[:, s * js : (s + 1) * js, :],
                            in_=x_dram[:, s * js : (s + 1) * js, :],
                        )
                    else:
                        j0, c0 = (s * sub) // cols, (s * sub) % cols
                        nc.sync.dma_start(
                            out=x_t3[:, j0, c0 : c0 + sub],
                            in_=x_dram[:, j0, c0 : c0 + sub],
                        )
                for s in range(sub_splits):
                    xs = x_t[:, s * sub : (s + 1) * sub]
                    xb = xs.rearrange("p (b i) -> p b i", i=BLOCK)
                    nblk = sub // BLOCK
                    # maxabs per 64-block: one fused |.|+max reduce
                    m = sp.tile([P, nblk], mybir.dt.float32, name="m", tag="m", bufs=4)
                    nc.vector.tensor_reduce(
                        out=m,
                        in_=xb,
                        axis=mybir.AxisListType.X,
                        op=mybir.AluOpType.max,
                        apply_absolute_value=True,
                    )
                    # delta_bits = (bits(maxabs) & EXP_MASK) - (6 << 23)
                    # (walrus rejects bitwise+arith in one tensor_scalar)
                    db = sp.tile([P, nblk], mybir.dt.int32, name="db", tag="db", bufs=4)
                    nc.vector.tensor_scalar(
                        out=db,
                        in0=m.bitcast(mybir.dt.int32),
                        scalar1=_EXP_MASK,
                        scalar2=None,
                        op0=mybir.AluOpType.bitwise_and,
                    )
                    nc.vector.tensor_scalar(
                        out=db,
                        in0=db,
                        scalar1=_DELTA_BIAS,
                        scalar2=None,
                        op0=mybir.AluOpType.subtract,
                    )
                    # inv_bits = 0x7F000000 - delta_bits  (== bits of 1/delta)
                    ib = sp.tile([P, nblk], mybir.dt.int32, name="ib", tag="ib", bufs=4)
                    nc.vector.tensor_scalar(
                        out=ib,
                        in0=db,
                        scalar1=-1,
                        scalar2=_INV_CONST,
                        op0=mybir.AluOpType.mult,
                        op1=mybir.AluOpType.add,
                    )
                    inv_b = (
                        ib.bitcast(mybir.dt.float32)
                        .unsqueeze(2)
                        .broadcast_to((P, nblk, BLOCK))
                    )
                    delta_b = (
                        db.bitcast(mybir.dt.float32)
                        .unsqueeze(2)
                        .broadcast_to((P, nblk, BLOCK))
                    )
                    # y = x / delta (exact)
                    nc.vector.tensor_tensor(
                        out=xb, in0=xb, in1=inv_b, op=mybir.AluOpType.mult
                    )
                    # r = RNE(y)
                    nc.vector.tensor_scalar(
                        out=xs,
                        in0=xs,
                        scalar1=_RNE_C,
                        scalar2=_RNE_C,
                        op0=mybir.AluOpType.add,
                        op1=mybir.AluOpType.subtract,
                    )
                    # q = min(r, 127) * delta
                    qs = q[:, s * sub : (s + 1) * sub]
                    qb = qs.rearrange("p (b i) -> p b i", i=BLOCK)
                    nc.vector.scalar_tensor_tensor(
                        out=qb,
                        in0=xb,
                        scalar=127.0,
                        in1=delta_b,
                        op0=mybir.AluOpType.min,
                        op1=mybir.AluOpType.mult,
                    )
                out_dram = out[r0 : r0 + P * jt, :].rearrange("(j p) c -> p j c", p=P)
                nc.sync.dma_start(
                    out=out_dram, in_=q.rearrange("p (j c) -> p j c", j=jt)
                )
            if rep_ctx is not None:
                rep_ctx.__exit__(None, None, None)
    nc.compile()
    return nc




def build_quant_v3(
    rows: int = SHARD_ROWS,
    cols: int = COLS,
    reps: int = 1,
    xp_bufs: int = 2,
    gpsimd_tt2: bool = False,
):
    """v3: per-tile [128, 8192] pipeline.
    - ACT extracts |trunc_bf16(x)| from the fp32 high halves (exponent-exact,
      truncation is monotone -> exponent(max) is preserved).
    - DVE folds a bf16 max tree (2x mode) to per-block maxima.
    - delta/inv derived bit-exactly from the exponent field (int32 ops).
    - TT1: y8 = x * inv -> int8 output (HW-verified RNE + saturation does the
      round AND the clip to [-128, 127] in the cast).
    - TT2: q = y8 * delta -> bf16 (exact).
    """
    nc = bacc.Bacc("TRN2")
    x = nc.dram_tensor("x", [rows, cols], mybir.dt.float32, kind="ExternalInput")
    out = nc.dram_tensor("out", [rows, cols], mybir.dt.bfloat16, kind="ExternalOutput")
    n_tiles = rows // P
    nblk = cols // BLOCK

    with tile.TileContext(nc) as tc:
        with (
            tc.tile_pool(name="xp", bufs=xp_bufs) as xp,
            tc.tile_pool(name="qp", bufs=2) as qp,
            tc.tile_pool(name="hp", bufs=2) as hp,
            tc.tile_pool(name="tp", bufs=2) as tp,
            tc.tile_pool(name="sp", bufs=4) as sp,
        ):
            rep_ctx = tc.For_i(0, reps, 1) if reps > 1 else None
            if rep_ctx is not None:
                rep_ctx.__enter__()
            for t in range(n_tiles):
                x_t = xp.tile([P, cols], mybir.dt.float32, name="x_t", tag="x_t")
                nc.sync.dma_start(out=x_t, in_=x[t * P : (t + 1) * P, :])
                xb = x_t.rearrange("p (b i) -> p b i", i=BLOCK)

                # |trunc_bf16(x)|: strided high halves, Abs on ACT
                habs = hp.tile([P, cols], mybir.dt.bfloat16, name="habs", tag="habs")
                nc.scalar.activation(
                    out=habs,
                    in_=x_t.bitcast(mybir.dt.bfloat16)[:, 1::2],
                    func=mybir.ActivationFunctionType.Abs,
                )
                # bf16 max tree: 64 -> 32 -> 16 -> 8 -> 4 -> 2 -> 1 per block
                h3 = habs.rearrange("p (b i) -> p b i", i=BLOCK)
                t1 = tp.tile([P, nblk, 32], mybir.dt.bfloat16, name="t1", tag="t1")
                t2 = tp.tile([P, nblk, 16], mybir.dt.bfloat16, name="t2", tag="t2")
                m = sp.tile([P, nblk], mybir.dt.bfloat16, name="m", tag="m")
                mx = mybir.AluOpType.max
                nc.vector.tensor_tensor(out=t1, in0=h3[:, :, 0:32], in1=h3[:, :, 32:64], op=mx)
                nc.vector.tensor_tensor(out=t2, in0=t1[:, :, 0:16], in1=t1[:, :, 16:32], op=mx)
                nc.vector.tensor_tensor(out=t1[:, :, 0:8], in0=t2[:, :, 0:8], in1=t2[:, :, 8:16], op=mx)
                nc.vector.tensor_tensor(out=t2[:, :, 0:4], in0=t1[:, :, 0:4], in1=t1[:, :, 4:8], op=mx)
                nc.vector.tensor_tensor(out=t1[:, :, 0:2], in0=t2[:, :, 0:2], in1=t2[:, :, 2:4], op=mx)
                nc.vector.tensor_tensor(
                    out=m.rearrange("p (b i) -> p b i", i=1),
                    in0=t1[:, :, 0:1], in1=t1[:, :, 1:2], op=mx,
                )

                # delta/inv from the exponent field (proven int32 path)
                mf = sp.tile([P, nblk], mybir.dt.float32, name="mf", tag="mf")
                nc.vector.tensor_copy(out=mf, in_=m)
                db = sp.tile([P, nblk], mybir.dt.int32, name="db", tag="db")
                nc.vector.tensor_scalar(
                    out=db, in0=mf.bitcast(mybir.dt.int32),
                    scalar1=_EXP_MASK, scalar2=None, op0=mybir.AluOpType.bitwise_and,
                )
                nc.vector.tensor_scalar(
                    out=db, in0=db,
                    scalar1=_DELTA_BIAS, scalar2=None, op0=mybir.AluOpType.subtract,
                )
                ib = sp.tile([P, nblk], mybir.dt.int32, name="ib", tag="ib")
                nc.vector.tensor_scalar(
                    out=ib, in0=db, scalar1=-1, scalar2=_INV_CONST,
                    op0=mybir.AluOpType.mult, op1=mybir.AluOpType.add,
                )
                inv_b = (
                    ib.bitcast(mybir.dt.float32).unsqueeze(2)
                    .broadcast_to((P, nblk, BLOCK))
                )
                delta_b = (
                    db.bitcast(mybir.dt.float32).unsqueeze(2)
                    .broadcast_to((P, nblk, BLOCK))
                )
                # y8 = round/clip(x / delta) via saturating int8 cast;
                # habs is dead past the tree -> reuse its bytes for y8
                y8 = habs.bitcast(mybir.dt.int8)[:, 0:cols]
                y8b = y8.rearrange("p (b i) -> p b i", i=BLOCK)
                nc.vector.tensor_tensor(out=y8b, in0=xb, in1=inv_b, op=mybir.AluOpType.mult)
                # q = y8 * delta, exact in bf16
                q = qp.tile([P, cols], mybir.dt.bfloat16, name="q", tag="q")
                qb = q.rearrange("p (b i) -> p b i", i=BLOCK)
                eng2 = nc.gpsimd if gpsimd_tt2 else nc.vector
                eng2.tensor_tensor(out=qb, in0=y8b, in1=delta_b, op=mybir.AluOpType.mult)
                nc.sync.dma_start(out=out[t * P : (t + 1) * P, :], in_=q)
            if rep_ctx is not None:
                rep_ctx.__exit__(None, None, None)
    nc.compile()
    return nc


_nc_cache = {}


def _get_nc():
    if "nc" not in _nc_cache:
        _nc_cache["nc"] = build_quant_v3(xp_bufs=3)
    return _nc_cache["nc"]


def _run(x_np: np.ndarray, trace: bool = False):
    nc = _get_nc()
    shards = [
        np.ascontiguousarray(x_np[i * SHARD_ROWS : (i + 1) * SHARD_ROWS])
        for i in range(N_CORES)
    ]
    res = bass_utils.run_bass_kernel_spmd(
        nc,
        [{"x": s} for s in shards],
        core_ids=list(range(N_CORES)),
        trace=trace,
    )
    out = np.concatenate(
        [np.asarray(r["out"]).astype(np.float32) for r in res.results], axis=0
    )
    return out, res


def kernel(x, bits):
    assert int(np.asarray(bits)) == 8
    x_np = np.asarray(x, dtype=np.float32)
    assert x_np.shape == (ROWS, COLS)
    out, _ = _run(x_np, trace=False)
    return out


def bench_hw_ns(x_np, r_lo=1, r_hi=5001, n_times=6):
    """Estimate HW exec time of one kernel invocation by on-device repeat
    loops: wall(r_hi reps) - wall(r_lo reps) removes the constant axon RPC +
    host<->device transfer overhead. Returns (ns_per_iter, details)."""
    import time

    shards = [
        np.ascontiguousarray(x_np[i * SHARD_ROWS : (i + 1) * SHARD_ROWS])
        for i in range(N_CORES)
    ]
    in_maps = [{"x": s} for s in shards]
    walls = {}
    for reps in (r_lo, r_hi):
        nc = build_quant_v3(xp_bufs=3, reps=reps) if reps > 1 else _get_nc()
        ts = []
        for it in range(n_times):
            t0 = time.monotonic()
            bass_utils.run_bass_kernel_spmd(
                nc, in_maps, core_ids=list(range(N_CORES))
            )
            ts.append(time.monotonic() - t0)
        walls[reps] = sorted(ts)
    # walls are sorted; min is the most contention-robust estimator on the
    # shared axon terminal (first run of each config includes compile and
    # lands at the sorted tail)
    lo = walls[r_lo][0]
    hi = walls[r_hi][0]
    ns = (hi - lo) / (r_hi - r_lo) * 1e9
    return ns, walls
